# revision 12
# baseline (speedup 1.0000x reference)
"""K-means nearest-centroid assignment on Trainium2, data-parallel across 8 cores.

Reference computes argmin_k ||x_n - c_k||^2 == argmax_k (x_n . c_k - 0.5*||c_k||^2).
Each core gets N/8 points (transposed on host so the contraction dim C lands on
SBUF partitions); the centroid table is replicated.

Device pipeline per 128-point subtile:
  - PE: 16 fp16 matmuls accumulate x.cT into PSUM [128, 2048] (fp32 accum), then
    one 2-contraction-row matmul per 512-wide K-tile adds the centroid-norm bias
    as an fp16 hi+lo pair (error ~1e-5, far below the fp16 score noise).
  - DVE: max gives each point's top-8 scores (m8[0] winner, m8[0]-m8[1] the true
    top-2 margin); max_index finds the winner's position.
  - Host: k = i8[0]; points whose top-2 margin is below tau (fp16 rounding can
    flip near-ties, sigma~0.007 vs mean gap ~6.8) are re-solved exactly in
    numpy. Residual flips after the fixup are ~0, far inside the 2e-2 gate.
"""

import sys

sys.path.insert(0, "/opt/trn_rl_repo")

import numpy as np

import concourse.bass as bass
import concourse.bacc as bacc
import concourse.mybir as mybir
from concourse.tile import TileContext

N, C, K = 131072, 512, 2048
NCORES = 8
P = 128
KT = 512              # psum bank width in fp32 / matmul max moving dim
NKT = K // KT         # 4 K-tiles
NCC = C // P          # 4 contraction chunks
ST = 1024             # points per supertile (xT DMA free dim)
TAU = 0.05            # host fixup margin threshold (~7.5 sigma of fp16 noise)

F32 = mybir.dt.float32
F16 = mybir.dt.float16
MM_DT = F16


def build_nc(nloc, mm_dt=MM_DT):
    """One SPMD program: nloc points per core, full K centroids."""
    nsuper = nloc // ST
    nsub = ST // P

    nc = bacc.Bacc(None, target_bir_lowering=False)
    xT = nc.declare_dram_parameter("xT", [C, nloc], mm_dt, isOutput=False)
    cT = nc.declare_dram_parameter("cT", [C, K], mm_dt, isOutput=False)
    # bias2[0] = fp16 hi part of -0.5*||c_k||^2, bias2[1] = fp16 lo residual;
    # summed into every psum row by a 2-row ones matmul.
    bias2 = nc.declare_dram_parameter("bias2", [2, K], mm_dt, isOutput=False)
    out_m = nc.declare_dram_parameter("out_m", [nloc, 2], F32, isOutput=True)
    out_i = nc.declare_dram_parameter(
        "out_i", [nloc], mybir.dt.uint32, isOutput=True
    )

    with TileContext(nc) as tc:
        with (
            tc.tile_pool(name="const", bufs=1) as const_pool,
            tc.tile_pool(name="xin", bufs=4) as xin_pool,
            tc.tile_pool(name="res", bufs=8) as res_pool,
            tc.tile_pool(name="psum", bufs=2, space="PSUM") as psum_pool,
        ):
            cT_tiles = []
            for c in range(NCC):
                t = const_pool.tile([P, K], mm_dt, tag=f"cT{c}")
                nc.sync.dma_start(out=t[:], in_=cT[c * P:(c + 1) * P, :])
                cT_tiles.append(t)
            bias_t = const_pool.tile([2, K], mm_dt, tag="bias2")
            nc.sync.dma_start(out=bias_t[:], in_=bias2[:, :])
            ones2 = const_pool.tile([2, P], mm_dt, tag="ones2")
            nc.vector.memset(ones2[:], 1.0)

            for st in range(nsuper):
                n0 = st * ST
                x_tiles = []
                for c in range(NCC):
                    t = xin_pool.tile([P, ST], mm_dt, tag=f"x{c}")
                    nc.sync.dma_start(
                        out=t[:], in_=xT[c * P:(c + 1) * P, n0:n0 + ST]
                    )
                    x_tiles.append(t)
                for s in range(nsub):
                    ps = psum_pool.tile([P, K], mybir.dt.float32, tag="ps")
                    for c in range(NCC):
                        for j in range(NKT):
                            nc.tensor.matmul(
                                ps[:, j * KT:(j + 1) * KT],
                                lhsT=x_tiles[c][:, s * P:(s + 1) * P],
                                rhs=cT_tiles[c][:, j * KT:(j + 1) * KT],
                                start=(c == 0),
                                stop=False,
                            )
                    for j in range(NKT):
                        nc.tensor.matmul(
                            ps[:, j * KT:(j + 1) * KT],
                            lhsT=ones2[:],
                            rhs=bias_t[:, j * KT:(j + 1) * KT],
                            start=False,
                            stop=True,
                        )
                    m8 = res_pool.tile([P, 8], mybir.dt.float32, tag="m8")
                    i8 = res_pool.tile([P, 8], mybir.dt.uint32, tag="i8")
                    nc.vector.max(m8[:], ps[:])
                    nc.vector.max_index(i8[:], m8[:], ps[:])
                    rows = slice(n0 + s * P, n0 + (s + 1) * P)
                    nc.gpsimd.dma_start(out=out_m[rows, :], in_=m8[:, 0:2])
                    nc.gpsimd.dma_start(out=out_i[rows], in_=i8[:, 0:1])
    nc.finalize()
    return nc


def make_in_maps(inp, centroids, nloc=None, ncores=NCORES):
    inp = np.asarray(inp, dtype=np.float32)
    centroids = np.asarray(centroids, dtype=np.float32)
    if nloc is None:
        nloc = inp.shape[0] // ncores
    cT = np.ascontiguousarray(centroids.T.astype(np.float16))
    c2 = np.sum(centroids.astype(np.float64) ** 2, axis=1)
    bias_row = -0.5 * c2
    bh = bias_row.astype(np.float16)
    bl = (bias_row - bh.astype(np.float64)).astype(np.float16)
    bias2 = np.ascontiguousarray(np.stack([bh, bl], axis=0))
    in_maps = []
    for i in range(ncores):
        xl = inp[i * nloc:(i + 1) * nloc]
        in_maps.append(
            {
                "xT": np.ascontiguousarray(xl.T.astype(np.float16)),
                "cT": cT,
                "bias2": bias2,
            }
        )
    return in_maps


def postprocess(m_all, i_all, inp, centroids, tau=TAU):
    """k from the device argmax; exact numpy re-solve for low-margin points
    (fp16 near-ties) and any unmatched (-1) indices."""
    inp = np.asarray(inp, dtype=np.float32)
    centroids = np.asarray(centroids, dtype=np.float32)
    k = i_all.reshape(-1).astype(np.int64)
    margin = m_all[:, 0] - m_all[:, 1]
    bad = (margin < tau) | (k >= K)
    idx = np.nonzero(bad)[0]
    if idx.size:
        c2 = np.sum(centroids * centroids, axis=1)
        for a in range(0, idx.size, 8192):
            sel = idx[a:a + 8192]
            xb = inp[sel]
            d2 = (
                np.sum(xb * xb, axis=1, keepdims=True)
                - 2.0 * (xb @ centroids.T)
                + c2[None, :]
            )
            k[sel] = np.argmin(d2, axis=1)
    return k


def kernel(inp, centroids):
    from concourse.bass_utils import run_bass_kernel_spmd

    nloc = N // NCORES
    nc = build_nc(nloc)
    in_maps = make_in_maps(inp, centroids, nloc=nloc)
    res = run_bass_kernel_spmd(nc, in_maps, core_ids=list(range(NCORES)))
    m_all = np.concatenate(
        [res.results[i]["out_m"].reshape(nloc, 2) for i in range(NCORES)]
    )
    i_all = np.concatenate(
        [res.results[i]["out_i"].reshape(-1) for i in range(NCORES)]
    )
    k = postprocess(m_all, i_all, inp, centroids)
    return k.astype(np.int32)
